# revision 19
# baseline (speedup 1.0000x reference)
"""CalderaLinear fused kernel for 8 Trainium2 NeuronCores (fp8 DoubleRow).

Math (reference): y = x @ Q^T + (x @ R^T) @ L^T + bias, with Q/L/R groupwise
int-dequantized (codes 0..15, group size 128 along the contraction dim).

Strategy (token-parallel / data-parallel):
  * Core c owns tokens [c*1024, (c+1)*1024) and computes its full y rows;
    outputs concatenate along axis 0. No replicated FLOPs, no collectives.
  * Error structure: the low-rank term dominates ||y|| by ~80x (xr entries
    have std ~325, amplified again through L), so base-path errors are
    diluted ~80x. The big base matmul (x @ Q^T, 275 of 310 GFLOP) therefore
    runs in fp8 e4m3 with MatmulPerfMode.DoubleRow (2 k-planes of 128
    contracted per pass = 2x bf16 PE throughput); int codes 0..15 are exact
    in e4m3, x and the code*scale products round at ~3% which lands ~4e-4
    on the output. The precision-critical low-rank path (xr = x @ R^T, then
    xr @ L^T) stays bf16, keeping total rel err ~3e-3 like the bf16 kernel.
  * All dequantization happens on-device (DVE): codes arrive as exact
    fp8/bf16 values, scales arrive pre-broadcast along partitions; a DVE
    multiply produces dequantized weights (in-place for Q/R/L).
  * Per core: prologue computes xr^T = R @ x^T on the PE (bf16, out-features
    of xr on PSUM partitions so no transpose is ever needed) and casts x to
    fp8; the main loop walks 4 out-feature quarters x 8 token groups, each
    PSUM group = 2 bf16 low-rank matmuls + 16x4 fp8 DoubleRow base matmuls,
    bias fused into the DVE eviction.
  * Q codes+scales (16.8 MB each, fp8) stream through a 2-quarter SBUF ring
    overlapped with compute; x streams per k-tile in the prologue.

PE budget/core: 65K cyc (xr^T) + 65K (low-rank) + 524K (base fp8) = 655K
cyc = 273 us at 2.4 GHz, vs 1.05M cyc (437 us) for the all-bf16 kernel.
"""

import numpy as np
import ml_dtypes

P = 128
D_IN = 4096
D_OUT = 4096
TOK = 8192
RANK = 256
NCORES = 8
TPC = TOK // NCORES       # 1024 tokens per core
KT = D_IN // P            # 32 k-tiles
K2 = KT // 2              # 16 double-k-tiles (DoubleRow contracts 256)
NQ = 4                    # out-feature quarters
QW = D_OUT // NQ          # 1024
OCC = 512                 # psum chunk width (one bank)
TG = TPC // P             # 8 token groups
RT = RANK // P            # 2 rank tiles

_module_cache = {}
last_result = None


def _build_module():
    import concourse.mybir as mybir
    import concourse.tile as tile
    from concourse import bacc

    f32 = mybir.dt.float32
    bf16 = mybir.dt.bfloat16
    fp8 = mybir.dt.float8e4
    DR = mybir.MatmulPerfMode.DoubleRow

    nc = bacc.Bacc(None, target_bir_lowering=False, debug=False)
    # chunked layouts are partition-major within each chunk so one DMA
    # fills an SBUF tile slice with matching AP iteration order
    xb_d = nc.dram_tensor("xb", (KT // 4, P, 4, TPC), bf16,
                          kind="ExternalInput")
    qv_d = nc.dram_tensor("qv", (NQ, K2 // 4, P, 4, 2, QW), fp8,
                          kind="ExternalInput")
    qs_d = nc.dram_tensor("qs", (NQ, K2 // 4, P, 4, 2, QW), fp8,
                          kind="ExternalInput")
    rv_d = nc.dram_tensor("rv", (P, KT, RANK), bf16, kind="ExternalInput")
    rs_d = nc.dram_tensor("rs", (P, KT, RANK), bf16, kind="ExternalInput")
    lv_d = nc.dram_tensor("lv", (P, RT, D_OUT), bf16, kind="ExternalInput")
    ls_d = nc.dram_tensor("ls", (P, RT, D_OUT), bf16, kind="ExternalInput")
    bias_d = nc.dram_tensor("biasv", (P, D_OUT), f32, kind="ExternalInput")
    y_d = nc.dram_tensor("y", (TPC, D_OUT), f32, kind="ExternalOutput")

    with tile.TileContext(nc) as tc:
        with (
            tc.tile_pool(name="const", bufs=1) as const,
            tc.tile_pool(name="xbp", bufs=2) as xbp,
            tc.tile_pool(name="qp", bufs=2) as qp,
            tc.tile_pool(name="qsp", bufs=2) as qsp,
            tc.tile_pool(name="yp", bufs=4) as yp,
            tc.tile_pool(name="xrps", bufs=4, space="PSUM") as xrps,
            tc.tile_pool(name="pp", bufs=4, space="PSUM") as pp,
        ):
            rv = const.tile([P, KT, RANK], bf16)    # becomes dequantized R^T
            lv = const.tile([P, RT, D_OUT], bf16)   # becomes dequantized L^T
            bias_t = const.tile([P, D_OUT], f32)
            xrT = const.tile([P, RT, TPC], bf16)    # xr^T: [rank, tokens]
            x8 = const.tile([P, K2, 2, TPC], fp8)
            # prologue scratch staged in the Q-quarter ring (slots are
            # reclaimed by quarters 0/1 once the scales are consumed)
            rs = qp.tile([P, KT, RANK], bf16, tag="q", name="rs")
            ls = qp.tile([P, RT, D_OUT], bf16, tag="q", name="ls")

            # R goes on the fast HW queue (gates the first xr matmuls);
            # L/bias are not needed until ~35us in, so they ride the slow
            # gpsimd SW queue, keeping the HW queues clear for x and Q.
            nc.sync.dma_start(rv[:], rv_d[:])
            nc.scalar.dma_start(rs[:], rs_d[:])
            nc.gpsimd.dma_start(lv[:], lv_d[:])
            nc.gpsimd.dma_start(ls[:], ls_d[:])
            nc.gpsimd.dma_start(bias_t[:], bias_d[:])

            nc.vector.tensor_mul(rv[:], rv[:], rs[:])
            nc.vector.tensor_mul(lv[:], lv[:], ls[:])

            # quarter tiles allocated up-front in ring order (rs, ls, q0..q3);
            # DMAs/dequants are emitted interleaved with compute below.
            qts = [
                qp.tile([P, K2, 2, QW], fp8, tag="q", name=f"q{q}")
                for q in range(NQ)
            ]
            qsts = {}

            def fetch_chunk(q, c):
                # one 1MB DMA per 4 k2-tiles of codes and of scales
                nc.sync.dma_start(qts[q][:, 4 * c:4 * c + 4], qv_d[q, c])
                qst = qsp.tile([P, 4, 2, QW], fp8, tag="qs", name=f"qs{q}_{c}")
                nc.scalar.dma_start(qst[:], qs_d[q, c])
                qsts[(q, c)] = qst

            def dequant(q, k2, eng=None):
                (eng or nc.vector).tensor_mul(
                    qts[q][:, k2], qts[q][:, k2], qsts[(q, k2 // 4)][:, k2 % 4]
                )

            # ---- prologue: xr^T = R @ x^T (bf16) + cast x -> fp8 (on Act)
            xr_ps = [
                xrps.tile([P, OCC], f32, tag="xr", name=f"xr{i}")
                for i in range(4)
            ]
            for c in range(KT // 4):
                xbt = xbp.tile([P, 4, TPC], bf16, tag="xb")
                dma_x = nc.sync if c % 2 == 0 else nc.scalar
                dma_x.dma_start(xbt[:], xb_d[c])
                # interleave quarter-0 Q chunks into the x stream so the
                # dequants can start well before the main loop begins
                if c % 2 == 0:
                    fetch_chunk(0, c // 2)
                for kk in range(4):
                    k = 4 * c + kk
                    for ts in range(2):
                        for rt in range(RT):
                            nc.tensor.matmul(
                                xr_ps[rt * 2 + ts][:],
                                rv[:, k, rt * P:(rt + 1) * P],
                                xbt[:, kk, ts * OCC:(ts + 1) * OCC],
                                start=(k == 0), stop=(k == KT - 1),
                            )
                # chunk c covers k2 = 2c, 2c+1: same (k2, pl, tok) order
                nc.scalar.copy(x8[:, 2 * c:2 * c + 2], xbt[:])
            # quarter-0 dequants: mostly DVE, tail on the idle gpsimd; the
            # xr^T evictions slot in mid-stream (the PE needs xrT only ~7us
            # into the main loop)
            for k2 in range(6):
                dequant(0, k2)
            for rt in range(RT):
                for ts in range(2):
                    nc.vector.tensor_copy(
                        xrT[:, rt, ts * OCC:(ts + 1) * OCC],
                        xr_ps[rt * 2 + ts][:],
                    )
            for k2 in range(6, 13):
                dequant(0, k2)
            for k2 in range(13, K2):
                dequant(0, k2, nc.gpsimd)

            # ---- main: 4 quarters x 8 token groups; base matmuls first in
            # each psum group (start), low-rank last (stop); quarter q+1 is
            # fetched/dequanted two k2-tiles per token group so the in-order
            # DVE stays ahead of the PE.
            for q in range(NQ):
                for t in range(TG):
                    psums = [
                        pp.tile([P, OCC], f32, tag="ps", name=f"ps{q}_{t}_{i}")
                        for i in range(2)
                    ]
                    for k2 in range(K2):
                        for oc2 in range(2):
                            nc.tensor.matmul(
                                psums[oc2][:],
                                x8[:, k2, :, t * P:(t + 1) * P],
                                qts[q][:, k2, :, oc2 * OCC:(oc2 + 1) * OCC],
                                start=(k2 == 0), stop=False,
                                perf_mode=DR,
                            )
                    for oc2 in range(2):
                        for rt in range(RT):
                            nc.tensor.matmul(
                                psums[oc2][:],
                                xrT[:, rt, t * P:(t + 1) * P],
                                lv[:, rt, q * QW + oc2 * OCC:
                                   q * QW + (oc2 + 1) * OCC],
                                start=False, stop=(rt == RT - 1),
                            )
                    if q + 1 < NQ:
                        if t % 2 == 0:
                            fetch_chunk(q + 1, t // 2)
                        dequant(q + 1, 2 * t)
                        dequant(q + 1, 2 * t + 1, nc.gpsimd)
                    yt = yp.tile([P, QW], f32, tag="y")
                    for oc2 in range(2):
                        nc.vector.tensor_add(
                            yt[:, oc2 * OCC:(oc2 + 1) * OCC], psums[oc2][:],
                            bias_t[:, q * QW + oc2 * OCC:
                                   q * QW + (oc2 + 1) * OCC],
                        )
                    dma_y = nc.sync if t % 2 == 0 else nc.scalar
                    dma_y.dma_start(
                        y_d[t * P:(t + 1) * P, q * QW:(q + 1) * QW], yt[:]
                    )

    nc.compile()
    return nc


def kernel(x, q_values, q_scales, l_values, l_scales, r_values, r_scales, bias,
           _trace=False):
    from concourse.bass_utils import run_bass_kernel_spmd

    if "mod" not in _module_cache:
        _module_cache["mod"] = _build_module()
    nc = _module_cache["mod"]

    bf = ml_dtypes.bfloat16
    f8 = ml_dtypes.float8_e4m3
    x = np.asarray(x, np.float32)
    qv = np.asarray(q_values)
    qsc = np.asarray(q_scales, np.float32)
    lvv = np.asarray(l_values)
    lsc = np.asarray(l_scales, np.float32)
    rvv = np.asarray(r_values)
    rsc = np.asarray(r_scales, np.float32)
    b = np.asarray(bias, np.float32)

    # host-side marshaling (layout + dtype only; all dequant/matmul math
    # runs on-device). d_in index i = k2*256 + pl*128 + p throughout.
    # [q, c, p, kk, pl, oq] with k2 = 4c + kk, d_in i = k2*256 + pl*128 + p
    qv_h = np.ascontiguousarray(
        qv.T.reshape(K2 // 4, 4, 2, P, NQ, QW).transpose(4, 0, 3, 1, 2, 5)
    ).astype(np.float32).astype(f8)
    qs_small = (
        qsc.T.reshape(K2 // 4, 4, 2, NQ, QW).transpose(3, 0, 1, 2, 4).astype(f8)
    )
    qs_h = np.ascontiguousarray(
        np.broadcast_to(qs_small[:, :, None, :, :, :], (NQ, K2 // 4, P, 4, 2, QW))
    )
    rv_h = np.ascontiguousarray(
        rvv.T.reshape(KT, P, RANK).transpose(1, 0, 2)
    ).astype(np.float32).astype(bf)
    rs_h = np.ascontiguousarray(
        np.broadcast_to(rsc.T.astype(bf)[None, :, :], (P, KT, RANK))
    )
    lv_h = np.ascontiguousarray(
        lvv.T.reshape(RT, P, D_OUT).transpose(1, 0, 2)
    ).astype(np.float32).astype(bf)
    ls_h = np.ascontiguousarray(
        np.broadcast_to(lsc.T.astype(bf)[None, :, :], (P, RT, D_OUT))
    )
    bias_h = np.ascontiguousarray(np.broadcast_to(b[None, :], (P, D_OUT)))

    in_maps = []
    for c in range(NCORES):
        xb_h = np.ascontiguousarray(
            x[c * TPC:(c + 1) * TPC].T.reshape(KT // 4, 4, P, TPC)
            .transpose(0, 2, 1, 3)
        ).astype(bf)
        in_maps.append({
            "xb": xb_h, "qv": qv_h, "qs": qs_h, "rv": rv_h, "rs": rs_h,
            "lv": lv_h, "ls": ls_h, "biasv": bias_h,
        })

    res = run_bass_kernel_spmd(
        nc, in_maps, core_ids=list(range(NCORES)), trace=_trace
    )
    global last_result
    last_result = res
    return np.concatenate([r["y"] for r in res.results], axis=0)
